# revision 24
# baseline (speedup 1.0000x reference)
"""AsterHead decoder (attention GRU + big fc) on 8 TRN2 NeuronCores.

Strategy: pure data-parallel over batch (B=256 -> 32/core, no collectives).
Recurrent attention runs in a "transposed" layout (feature on partitions,
b-major (b*25+t) free axis) so reductions over t are contiguous and biases
are per-partition.  GRU gate matmuls run in NORMAL orientation (stationary
= state/context [128,32] chunks, moving = weight tiles N=512) which cuts
the per-step matmul count 4x; gates run on [32, G] tiles and the new state
is transposed back with 4 PE transposes.  All TensorE math is bf16
(rel-err gate is loose), gates/softmax accumulate in f32.  The big fc
[800,512]@[512,6625] and all loop-invariant projections are hoisted out of
the 25-step recurrence; the target-embedding path goes through a DRAM
scratch and is prefetched per step.
"""

import sys

for _p in ("/opt/trn_rl_repo",):
    if _p not in sys.path:
        sys.path.insert(0, _p)

from contextlib import ExitStack

import ml_dtypes
import numpy as np

import concourse.bass as bass
import concourse.tile as tile
from concourse import bacc, mybir
from concourse.bass_utils import run_bass_kernel_spmd
from concourse.masks import make_identity

B, T, X, S, A, Y, EMB = 256, 25, 512, 512, 512, 6625, 300
NCORES, BL = 8, 32
BT = T * BL            # 800
G = 3 * S              # 1536
KX, KS, KA = X // 128, S // 128, A // 128   # 4 each
KG = G // 128          # 12
EMBP = 384             # EMB padded to 3*128
NY = [512] * 12 + [Y - 12 * 512]   # fc N-splits (481 last)
NBT = ((0, 512), (512, 288))       # 800 -> 512 + 288

bf = mybir.dt.bfloat16
f32 = mybir.dt.float32
i32 = mybir.dt.int32
bfnp = ml_dtypes.bfloat16

TRACE = False
LAST_EXEC_NS = None
_CACHE = {}

AF = mybir.ActivationFunctionType
OP = mybir.AluOpType
AX = mybir.AxisListType


def _build():
    nc = bacc.Bacc("TRN2", target_bir_lowering=False, debug=False,
                   num_devices=NCORES)

    def dp(name, sh, dt):
        return nc.dram_tensor(name, list(sh), dt, kind="ExternalInput").ap()

    xT = dp("xT", (X, BT), bf)            # [x, b*25+t]
    xft = dp("xft", (T * X, BL), bf)      # [t*X+x, b]
    yidx = dp("yidx", (BT,), i32)         # b-major decoder ids
    eEw = dp("eEw", (T * X, EMB), bf)     # eEmbed_w.T
    wfc = dp("wfc", (EMBP, S), bf)        # embed_fc_w.T zero-padded
    sEw = dp("sEw", (S, A), bf)
    xEw = dp("xEw", (X, A), bf)
    wEw = dp("wEw", (A, 1), bf)
    temb = dp("temb", (Y + 1, A), bf)
    wihy = dp("wihy", (A, G), bf)
    wihc = dp("wihc", (X, G), bf)
    whh = dp("whh", (S, G), bf)
    fcw = dp("fcw", (S, Y), bf)
    fcb = dp("fcb", (Y,), bf)
    sb0 = dp("sb0", (S,), f32)
    sxb = dp("sxb", (A,), f32)
    bgiy = dp("bgiy", (G,), f32)
    bhn = dp("bhn", (S,), f32)
    out = nc.dram_tensor("out", [BL, T, Y], f32, kind="ExternalOutput").ap()
    dbg_st = nc.dram_tensor("dbg_st", [128, KS * T * BL], bf,
                            kind="ExternalOutput").ap()
    dbg_gy = nc.dram_tensor("dbg_gy", [BT, G], bf, kind="ExternalOutput").ap()

    with tile.TileContext(nc) as tc, ExitStack() as top:
        # ---------------- persistent (whole-kernel) pool -----------------
        pers = top.enter_context(tc.tile_pool(name="pers", bufs=1))
        dramp = top.enter_context(tc.tile_pool(name="dramp", bufs=1,
                                               space="DRAM"))
        giy_d = dramp.tile([BT, G], bf, tag="giy_d")
        ident = pers.tile([128, 128], bf, tag="ident")
        make_identity(nc, ident[:])
        ones1 = pers.tile([1, 128], bf, tag="ones1")
        nc.gpsimd.memset(ones1[:], 1.0)

        sxb_t = pers.tile([128, KA], f32, tag="sxb")
        nc.sync.dma_start(sxb_t[:], sxb.rearrange("(j p) -> p j", p=128))
        wEw_t = pers.tile([128, KA], bf, tag="wEw")
        nc.sync.dma_start(wEw_t.rearrange("p (j o) -> p j o", j=KA),
                          wEw.rearrange("(j p) o -> p j o", p=128))
        bhn1 = pers.tile([1, S], f32, tag="bhn1")
        nc.sync.dma_start(bhn1[:], bhn.rearrange("(o s) -> o s", o=1))
        bgiy1 = pers.tile([1, G], f32, tag="bgiy1")
        nc.sync.dma_start(bgiy1[:], bgiy.rearrange("(o g) -> o g", o=1))
        sb01 = pers.tile([1, S], f32, tag="sb01")
        nc.sync.dma_start(sb01[:], sb0.rearrange("(o s) -> o s", o=1))

        stN = pers.tile([32, S], f32, tag="stN")               # state (normal)
        stT_bf = pers.tile([128, KS * BL], bf, tag="stT_bf")   # transposed bf16
        states = pers.tile([128, KS * T * BL], bf, tag="states")  # [j, b*25+t]

        bhn1b = pers.tile([1, S], bf, tag="bhn1b")
        nc.vector.tensor_copy(bhn1b[:], bhn1[:])

        # ------------- mid-lifetime pool (phases 2..loop) -------------
        with ExitStack() as mid:
            midp = mid.enter_context(tc.tile_pool(name="midp", bufs=1))
            sEw_t = midp.tile([128, KS * A], bf, tag="sEw")
            nc.sync.dma_start(sEw_t.rearrange("p (j a) -> p j a", j=KS),
                              sEw.rearrange("(j p) a -> p j a", p=128))
            wihc_t = midp.tile([128, KX * G], bf, tag="wihc")
            nc.sync.dma_start(wihc_t.rearrange("p (j g) -> p j g", j=KX),
                              wihc.rearrange("(j p) g -> p j g", p=128))
            whh_t = midp.tile([128, KS * G], bf, tag="whh")
            nc.sync.dma_start(whh_t.rearrange("p (j g) -> p j g", j=KS),
                              whh.rearrange("(j p) g -> p j g", p=128))
            xT_sb = [midp.tile([128, BT], bf, tag=f"xT{j}", name=f"xT{j}")
                     for j in range(KX)]
            for j in range(KX):
                nc.sync.dma_start(xT_sb[j][:], xT[j * 128:(j + 1) * 128, :])
            xP_sb = [midp.tile([128, BT], bf, tag=f"xP{j}", name=f"xP{j}")
                     for j in range(KA)]

            # ---------------- phase 1: embed -> state0 ----------------
            with ExitStack() as ph:
                p_xft = ph.enter_context(tc.tile_pool(name="p_xft", bufs=1))
                p_ew = ph.enter_context(tc.tile_pool(name="p_ew", bufs=6))
                p_ps = ph.enter_context(
                    tc.tile_pool(name="p_ps", bufs=1, space="PSUM"))
                p_tmp = ph.enter_context(tc.tile_pool(name="p_tmp", bufs=1))
                p_pst = ph.enter_context(
                    tc.tile_pool(name="p_pst", bufs=2, space="PSUM"))

                xft_t = p_xft.tile([128, 100 * BL], bf)
                nc.sync.dma_start(xft_t.rearrange("p (k b) -> p k b", k=100),
                                  xft.rearrange("(k p) b -> p k b", p=128))
                ps_em = p_ps.tile([32, EMB], f32, tag="em")
                for k in range(100):
                    ew = p_ew.tile([128, EMB], bf, tag="ew")
                    nc.sync.dma_start(ew[:], eEw[k * 128:(k + 1) * 128, :])
                    nc.tensor.matmul(ps_em[:], xft_t[:, k * 32:(k + 1) * 32],
                                     ew[:], start=(k == 0), stop=(k == 99))
                embed = p_tmp.tile([32, EMB], bf, tag="embed")
                nc.scalar.copy(embed[:], ps_em[:])
                embedT = p_tmp.tile([128, 3 * 32], bf, tag="embedT")
                nc.vector.memset(embedT[:], 0.0)
                for j in range(3):
                    cnt = min(128, EMB - j * 128)
                    pst = p_pst.tile([128, 32], bf, tag="pst")
                    nc.tensor.transpose(pst[:cnt, :],
                                        embed[:, j * 128:j * 128 + cnt],
                                        ident[:32, :32])
                    nc.scalar.copy(embedT[:cnt, j * 32:(j + 1) * 32],
                                   pst[:cnt, :])
                wfc_t = p_tmp.tile([128, 3 * S], bf, tag="wfc")
                nc.sync.dma_start(wfc_t.rearrange("p (j a) -> p j a", j=3),
                                  wfc.rearrange("(j p) a -> p j a", p=128))
                # state0 normal [32, S]; sb0 bias via PE ones-row accum
                sb01b = p_tmp.tile([1, S], bf, tag="sb01b")
                nc.vector.tensor_copy(sb01b[:], sb01[:])
                ps_s0 = p_ps.tile([32, S], f32, tag="s0")
                nc.tensor.matmul(ps_s0[:], ones1[:, :32], sb01b[:],
                                 start=True, stop=False, skip_group_check=True)
                for j in range(3):
                    nc.tensor.matmul(ps_s0[:],
                                     embedT[:, j * 32:(j + 1) * 32],
                                     wfc_t[:, j * S:(j + 1) * S],
                                     start=False, stop=(j == 2),
                                     skip_group_check=True)
                nc.scalar.copy(stN[:], ps_s0[:])
                stN_bf = p_tmp.tile([32, S], bf, tag="stN_bf")
                nc.scalar.copy(stN_bf[:], ps_s0[:])
                ps_t0 = p_pst.tile([128, KS * BL], bf, tag="t0")
                for j in range(KS):
                    nc.tensor.transpose(ps_t0[:, j * 32:(j + 1) * 32],
                                        stN_bf[:, j * 128:(j + 1) * 128],
                                        ident[:32, :32])
                nc.scalar.copy(stT_bf[:], ps_t0[:])

            # ---------------- phase 2: xProjT ----------------
            with ExitStack() as ph:
                p_w = ph.enter_context(tc.tile_pool(name="p_xw", bufs=1))
                p_ps = ph.enter_context(
                    tc.tile_pool(name="p_xps", bufs=2, space="PSUM"))
                xEw_t = p_w.tile([128, KX * A], bf)
                nc.sync.dma_start(xEw_t.rearrange("p (j a) -> p j a", j=KX),
                                  xEw.rearrange("(j p) a -> p j a", p=128))
                for m in range(KA):
                    ps = p_ps.tile([128, BT], f32, tag="xp")
                    for (n0, nn) in NBT:
                        for j in range(KX):
                            nc.tensor.matmul(
                                ps[:, n0:n0 + nn],
                                xEw_t[:, j * A + m * 128:j * A + (m + 1) * 128],
                                xT_sb[j][:, n0:n0 + nn],
                                start=(j == 0), stop=(j == KX - 1))
                    nc.scalar.activation(xP_sb[m][:], ps[:], AF.Identity,
                                         bias=sxb_t[:, m:m + 1])

            # ---------- phase 3: gather tgt emb -> giy (normal, DRAM) -----
            with ExitStack() as ph:
                p_idx = ph.enter_context(tc.tile_pool(name="p_idx", bufs=1))
                p_yp = ph.enter_context(tc.tile_pool(name="p_yp", bufs=3))
                p_w = ph.enter_context(tc.tile_pool(name="p_yw", bufs=1))
                p_gb = ph.enter_context(tc.tile_pool(name="p_gb", bufs=1))
                p_go = ph.enter_context(tc.tile_pool(name="p_go", bufs=2))
                p_pst = ph.enter_context(
                    tc.tile_pool(name="p_ypst", bufs=2, space="PSUM"))
                p_ps = ph.enter_context(
                    tc.tile_pool(name="p_yps", bufs=2, space="PSUM"))

                idx_t = p_idx.tile([128, 7], i32)
                for gch in range(7):
                    cnt = min(128, BT - gch * 128)
                    nc.sync.dma_start(
                        idx_t[:cnt, gch:gch + 1],
                        yidx.rearrange("(p o) -> p o", o=1)[
                            gch * 128:gch * 128 + cnt, :])
                wihy_t = p_w.tile([128, KA * G], bf)
                nc.sync.dma_start(wihy_t.rearrange("p (j g) -> p j g", j=KA),
                                  wihy.rearrange("(j p) g -> p j g", p=128))
                # bgiy replicated [128, G] for the free-axis bias add
                bgiy1b = p_gb.tile([1, G], bf, tag="bg1b")
                nc.vector.tensor_copy(bgiy1b[:], bgiy1[:])
                bgiyN = p_gb.tile([128, G], bf, tag="bgN")
                for gj in range(3):
                    pb = p_pst.tile([128, 512], f32, tag="pbb")
                    nc.tensor.matmul(pb[:], ones1[:, :],
                                     bgiy1b[:, gj * 512:(gj + 1) * 512],
                                     start=True, stop=True)
                    nc.vector.tensor_copy(bgiyN[:, gj * 512:(gj + 1) * 512],
                                          pb[:])
                for gch in range(7):
                    cnt = min(128, BT - gch * 128)
                    yp = p_yp.tile([128, A], bf, tag="yp")
                    nc.gpsimd.indirect_dma_start(
                        out=yp[:cnt, :], out_offset=None, in_=temb[:, :],
                        in_offset=bass.IndirectOffsetOnAxis(
                            ap=idx_t[:cnt, gch:gch + 1], axis=0))
                    # transpose the gathered rows, then normal-orientation MMs
                    ypT = p_go.tile([128, KA * 128], bf, tag="ypT")
                    for kj in range(KA):
                        pst = p_pst.tile([128, 128], bf, tag="ypst")
                        nc.tensor.transpose(pst[:, :cnt],
                                            yp[:cnt, kj * 128:(kj + 1) * 128],
                                            ident[:cnt, :cnt])
                        nc.scalar.copy(ypT[:, kj * 128:kj * 128 + cnt],
                                       pst[:, :cnt])
                    go = p_go.tile([128, G], f32, tag="go")
                    for gj in range(3):
                        ps = p_ps.tile([128, 512], f32, tag="gyps")
                        for kj in range(KA):
                            nc.tensor.matmul(
                                ps[:cnt, :],
                                ypT[:, kj * 128:kj * 128 + cnt],
                                wihy_t[:, kj * G + gj * 512:
                                       kj * G + (gj + 1) * 512],
                                start=(kj == 0), stop=(kj == KA - 1))
                        nc.vector.tensor_tensor(
                            go[:cnt, gj * 512:(gj + 1) * 512],
                            ps[:cnt, :],
                            bgiyN[:cnt, gj * 512:(gj + 1) * 512], op=OP.add)
                    gob = p_go.tile([128, G], bf, tag="gob")
                    nc.vector.tensor_copy(gob[:cnt, :], go[:cnt, :])
                    nc.sync.dma_start(giy_d[:][gch * 128:gch * 128 + cnt, :],
                                      gob[:cnt, :])

            # ---------------- recurrent loop ----------------
            with ExitStack() as ph:
                ps_sp_p = ph.enter_context(
                    tc.tile_pool(name="ps_sp", bufs=1, space="PSUM"))
                ps_ear_p = ph.enter_context(
                    tc.tile_pool(name="ps_ear", bufs=1, space="PSUM"))
                ps_ga_p = ph.enter_context(
                    tc.tile_pool(name="ps_ga", bufs=1, space="PSUM"))
                ps_gn_p = ph.enter_context(
                    tc.tile_pool(name="ps_gn", bufs=1, space="PSUM"))
                ps_fc_p = ph.enter_context(
                    tc.tile_pool(name="ps_fc", bufs=1, space="PSUM"))
                p_fcw = ph.enter_context(tc.tile_pool(name="p_fcw", bufs=1))
                p_fco = ph.enter_context(tc.tile_pool(name="p_fco", bufs=2))
                lp = ph.enter_context(tc.tile_pool(name="lp", bufs=2))
                thp = ph.enter_context(tc.tile_pool(name="thp", bufs=4))
                lps = ph.enter_context(tc.tile_pool(name="lps", bufs=2))
                mtp = ph.enter_context(tc.tile_pool(name="mtp", bufs=1))
                gp = ph.enter_context(tc.tile_pool(name="gp", bufs=1))
                gyp = ph.enter_context(tc.tile_pool(name="gyp", bufs=3))

                fcw_sb = [p_fcw.tile([128, Y], bf, tag=f"fcw{j}",
                                     name=f"fcw{j}") for j in range(KS)]
                for j in range(KS):
                    nc.sync.dma_start(fcw_sb[j][:],
                                      fcw[j * 128:(j + 1) * 128, :])
                fcb1 = p_fcw.tile([1, Y], bf, tag="fcb1")
                nc.sync.dma_start(fcb1[:], fcb.rearrange("(o y) -> o y", o=1))
                st_f = states.rearrange("p (j bt) -> p j bt", j=KS)
                fc_queue = []
                NYOFF = [0]
                for nn in NY:
                    NYOFF.append(NYOFF[-1] + nn)

                def emit_fc_chunks(k):
                    done = 0
                    while fc_queue and done < k:
                        t0, tw, os, ci = fc_queue[0]
                        cnt = BL * tw
                        y0, nn = NYOFF[ci], NY[ci]
                        ps = ps_fc_p.tile([128, 512], f32, tag="fps")
                        nc.tensor.matmul(ps[:cnt, :nn], ones1[:, :cnt],
                                         fcb1[:, y0:y0 + nn],
                                         start=True, stop=False,
                                         skip_group_check=True)
                        for kj in range(KS):
                            nc.tensor.matmul(
                                ps[:cnt, :nn],
                                st_f[:, kj, t0 * BL:t0 * BL + cnt],
                                fcw_sb[kj][:, y0:y0 + nn],
                                start=False, stop=(kj == KS - 1),
                                skip_group_check=True)
                        nc.scalar.copy(os[:cnt, y0:y0 + nn], ps[:cnt, :nn])
                        fc_queue[0][3] += 1
                        done += 1
                        if fc_queue[0][3] == len(NY):
                            for tau in range(tw):
                                nc.gpsimd.dma_start(
                                    out[:, t0 + tau, :],
                                    os[tau * BL:(tau + 1) * BL, :])
                            fc_queue.pop(0)

                for t in range(T):
                    # prefetch giy rows {b*25+t} (strided)
                    gy = gyp.tile([32, G], bf, tag="gy")
                    nc.sync.dma_start(
                        gy[:],
                        giy_d[:].rearrange("(b t) g -> b t g", t=T)[:, t, :])
                    # sProjT [a-chunk on cols]: 16 tiny MMs, transposed out
                    ps_sp = ps_sp_p.tile([128, KA * BL], f32, tag="sp")
                    for m in range(KA):
                        for j in range(KS):
                            nc.tensor.matmul(
                                ps_sp[:, m * 32:(m + 1) * 32],
                                sEw_t[:, j * A + m * 128:j * A + (m + 1) * 128],
                                stT_bf[:, j * 32:(j + 1) * 32],
                                start=(j == 0), stop=(j == KS - 1))
                    spT = lps.tile([128, KA * BL], bf, tag="spT")
                    nc.scalar.copy(spT[:], ps_sp[:])
                    # tanh(sProj + xProj), all-bf16 SBUF
                    ths = []
                    for m in range(KA):
                        ti = lp.tile([128, BT], bf, tag="ti")
                        nc.vector.tensor_tensor(
                            ti.rearrange("p (b t) -> p b t", t=T),
                            xP_sb[m].rearrange("p (b t) -> p b t", t=T),
                            spT[:, m * 32:(m + 1) * 32]
                                .rearrange("p (b o) -> p b o", o=1)
                                .to_broadcast([128, BL, T]),
                            op=OP.add)
                        th = thp.tile([128, BT], bf, tag="th")
                        nc.scalar.activation(th[:], ti[:], AF.Sigmoid,
                                             scale=2.0)
                        ths.append(th)
                    # e = w . tanh -> [1, 800]
                    ps_e = ps_ear_p.tile([1, BT], f32, tag="ear")
                    for (n0, nn) in NBT:
                        for m in range(KA):
                            nc.tensor.matmul(ps_e[:, n0:n0 + nn],
                                             wEw_t[:, m:m + 1],
                                             ths[m][:, n0:n0 + nn],
                                             start=(m == 0),
                                             stop=(m == KA - 1))
                    # softmax over t: UNNORMALIZED exp; 1/sum folded later
                    exb = lps.tile([1, BT], bf, tag="exb")
                    nc.scalar.activation(exb[:], ps_e[:], AF.Exp)
                    # broadcast exp weights to 128 partitions via PE
                    ps_ar = ps_ear_p.tile([128, BT], f32, tag="ear")
                    for (n0, nn) in NBT:
                        nc.tensor.matmul(ps_ar[:, n0:n0 + nn], ones1[:, :],
                                         exb[:, n0:n0 + nn],
                                         start=True, stop=True)
                    arb = lp.tile([128, BT], bf, tag="arb")
                    nc.scalar.copy(arb[:], ps_ar[:])
                    emit_fc_chunks(2)
                    # per-b sums + reciprocal on the replicated copy
                    sm = lps.tile([128, BL], bf, tag="sm")
                    with nc.allow_low_precision(reason="softmax sums, 2e-2 gate"):
                        nc.vector.reduce_sum(
                            sm[:], arb.rearrange("p (b t) -> p b t", t=T),
                            axis=AX.X)
                    rc = lps.tile([128, BL], f32, tag="rc")
                    nc.vector.reciprocal(rc[:], sm[:])
                    # context (unnormalized) -> normalize during bf16 cast
                    mt = mtp.tile([128, KX * BT], bf, tag="mt")
                    for xc in range(KX):
                        nc.vector.tensor_tensor(
                            mt[:, xc * BT:(xc + 1) * BT],
                            xT_sb[xc][:], arb[:], op=OP.mult)
                    ctxf = lps.tile([128, KX * BL], bf, tag="ctxf")
                    with nc.allow_low_precision(reason="ctx sums, 2e-2 gate"):
                        nc.vector.reduce_sum(
                            ctxf.rearrange("p (j b) -> p j b", j=KX),
                            mt.rearrange("p (j b t) -> p j b t", j=KX, t=T),
                            axis=AX.X)
                    ctxb = lps.tile([128, KX * BL], bf, tag="ctxb")
                    nc.vector.tensor_tensor(
                        ctxb.rearrange("p (j b) -> p j b", j=KX),
                        ctxf.rearrange("p (j b) -> p j b", j=KX),
                        rc.rearrange("p (o b) -> p o b", o=1)
                            .to_broadcast([128, KX, BL]),
                        op=OP.mult)
                    # gi+gh (normal orientation); gy and b_hh_n are
                    # accumulated into PSUM via identity/ones-row matmuls
                    ps_a = ps_ga_p.tile([32, 1024], f32, tag="ga")
                    ps_b = ps_gn_p.tile([32, 512], f32, tag="gb")
                    ps_c = ps_gn_p.tile([32, 512], f32, tag="gc")
                    for gj in range(2):
                        nc.tensor.matmul(ps_a[:, gj * 512:(gj + 1) * 512],
                                         ident[:32, :32],
                                         gy[:, gj * 512:(gj + 1) * 512],
                                         start=True, stop=False,
                                         skip_group_check=True)
                    nc.tensor.matmul(ps_b[:], ident[:32, :32], gy[:, 1024:],
                                     start=True, stop=False,
                                     skip_group_check=True)
                    nc.tensor.matmul(ps_c[:], ones1[:, :32], bhn1b[:],
                                     start=True, stop=False,
                                     skip_group_check=True)
                    for kj in range(KX):
                        last = kj == KX - 1
                        for gj in range(2):
                            nc.tensor.matmul(
                                ps_a[:, gj * 512:(gj + 1) * 512],
                                ctxb[:, kj * 32:(kj + 1) * 32],
                                wihc_t[:, kj * G + gj * 512:
                                   kj * G + (gj + 1) * 512],
                                start=False, stop=False,
                                skip_group_check=True)
                        nc.tensor.matmul(
                            ps_b[:],
                            ctxb[:, kj * 32:(kj + 1) * 32],
                            wihc_t[:, kj * G + 1024:kj * G + 1536],
                            start=False, stop=last, skip_group_check=True)
                        for gj in range(2):
                            nc.tensor.matmul(
                                ps_a[:, gj * 512:(gj + 1) * 512],
                                stT_bf[:, kj * 32:(kj + 1) * 32],
                                whh_t[:, kj * G + gj * 512:
                                   kj * G + (gj + 1) * 512],
                                start=False, stop=last,
                                skip_group_check=True)
                        nc.tensor.matmul(
                            ps_c[:],
                            stT_bf[:, kj * 32:(kj + 1) * 32],
                            whh_t[:, kj * G + 1024:kj * G + 1536],
                            start=False, stop=last, skip_group_check=True)
                    emit_fc_chunks(2)
                    # gates (normal layout, f32 accum)
                    rz = gp.tile([32, 1024], f32, tag="rz")
                    nc.scalar.activation(rz[:], ps_a[:], AF.Sigmoid)
                    cc = gp.tile([32, 512], f32, tag="cc")
                    nc.vector.tensor_tensor(cc[:], rz[:, :512], ps_c[:],
                                            op=OP.mult)
                    dd = gp.tile([32, 512], f32, tag="dd")
                    nc.vector.tensor_tensor(dd[:], cc[:], ps_b[:], op=OP.add)
                    nn_ = gp.tile([32, 512], f32, tag="nn")
                    nc.scalar.activation(nn_[:], dd[:], AF.Sigmoid, scale=2.0)
                    nc.vector.tensor_scalar(nn_[:], nn_[:], 2.0, -1.0,
                                            op0=OP.mult, op1=OP.add)
                    ee = gp.tile([32, 512], f32, tag="ee")
                    nc.vector.tensor_tensor(ee[:], stN[:], nn_[:],
                                            op=OP.subtract)
                    ff = gp.tile([32, 512], f32, tag="ff")
                    nc.vector.tensor_tensor(ff[:], rz[:, 512:], ee[:],
                                            op=OP.mult)
                    nc.vector.tensor_tensor(stN[:], nn_[:], ff[:], op=OP.add)
                    # new state: bf16 cast -> 4 PE transposes -> stT_bf
                    stN_bf = lps.tile([32, S], bf, tag="stN_bf")
                    nc.scalar.copy(stN_bf[:], stN[:])
                    ps_t = ps_sp_p.tile([128, KS * BL], bf, tag="sp")
                    for j in range(KS):
                        nc.tensor.transpose(ps_t[:, j * 32:(j + 1) * 32],
                                            stN_bf[:, j * 128:(j + 1) * 128],
                                            ident[:32, :32])
                    nc.scalar.copy(stT_bf[:], ps_t[:])
                    nc.vector.tensor_copy(
                        states.rearrange("p (j t b) -> p j t b",
                                         j=KS, t=T)[:, :, t, :],
                        stT_bf.rearrange("p (j b) -> p j b", j=KS))
                    # queue completed 4-step windows for interleaved fc
                    if t % 4 == 3 or t == T - 1:
                        t0 = t - 3 if t % 4 == 3 else t
                        tw = 4 if t % 4 == 3 else 1
                        os = p_fco.tile([128, Y], bf, tag="os")
                        fc_queue.append([t0, tw, os, 0])
                    if t == T - 1:
                        while fc_queue:
                            emit_fc_chunks(99)
                if t == T - 1:
                    nc.sync.dma_start(dbg_st[:, :], states[:])
                    nc.sync.dma_start(dbg_gy[:, :], giy_d[:])

    nc.compile()
    return nc


def _host_prep(inputs):
    x = np.asarray(inputs["x"], np.float32)
    tg = np.asarray(inputs["targets"]).astype(np.int64)
    assert int(inputs["lengths"]) == T, "kernel hardcodes T_dec=25"
    w = {k: np.asarray(v, np.float32) for k, v in inputs.items()
         if k not in ("x", "targets", "lengths")}

    y_seq = np.full((T, B), Y, np.int64)
    y_seq[1:] = tg[:, :T - 1].T
    y_seq = y_seq.astype(np.int32)          # [t, b_global]

    wfc_pad = np.zeros((EMBP, S), np.float32)
    wfc_pad[:EMB] = w["embed_fc_w"].T
    shared = {
        "eEw": np.ascontiguousarray(w["eEmbed_w"].T).astype(bfnp),
        "wfc": wfc_pad.astype(bfnp),
        "sEw": np.ascontiguousarray(w["sEmbed_w"].T).astype(bfnp),
        "xEw": np.ascontiguousarray(w["xEmbed_w"].T).astype(bfnp),
        "wEw": np.ascontiguousarray(2.0 * w["wEmbed_w"].T).astype(bfnp),
        "temb": np.ascontiguousarray(w["tgt_emb"]).astype(bfnp),
        "wihy": np.ascontiguousarray(w["gru_w_ih"][:, :A].T).astype(bfnp),
        "wihc": np.ascontiguousarray(w["gru_w_ih"][:, A:].T).astype(bfnp),
        "whh": np.ascontiguousarray(w["gru_w_hh"].T).astype(bfnp),
        "fcw": np.ascontiguousarray(w["fc_w"].T).astype(bfnp),
        "fcb": w["fc_b"].astype(bfnp),
        "sb0": (w["embed_fc_w"] @ w["eEmbed_b"] + w["embed_fc_b"]).astype(np.float32),
        "sxb": (w["sEmbed_b"] + w["xEmbed_b"]).astype(np.float32),
        "bgiy": (w["gru_b_ih"]
                 + np.concatenate([w["gru_b_hh"][:2 * S],
                                   np.zeros(S, np.float32)])).astype(np.float32),
        "bhn": w["gru_b_hh"][2 * S:].astype(np.float32),
    }
    in_maps = []
    for c in range(NCORES):
        xc = x[c * BL:(c + 1) * BL]                       # [32, 25, 512]
        m = dict(shared)
        m["xT"] = np.ascontiguousarray(
            xc.transpose(2, 0, 1).reshape(X, BT)).astype(bfnp)   # b-major
        m["xft"] = np.ascontiguousarray(
            xc.transpose(1, 2, 0).reshape(T * X, BL)).astype(bfnp)
        m["yidx"] = np.ascontiguousarray(
            y_seq[:, c * BL:(c + 1) * BL].T.reshape(BT))         # b-major
        in_maps.append(m)
    return in_maps


def kernel(**inputs):
    global LAST_EXEC_NS
    if "nc" not in _CACHE:
        _CACHE["nc"] = _build()
    nc = _CACHE["nc"]
    in_maps = _host_prep(inputs)
    res = run_bass_kernel_spmd(nc, in_maps, core_ids=list(range(NCORES)),
                               trace=TRACE)
    LAST_EXEC_NS = res.exec_time_ns
    outs = [res.results[c]["out"] for c in range(NCORES)]
    return np.concatenate(outs, axis=0)


# revision 25
# speedup vs baseline: 1.2925x; 1.2925x over previous
"""AsterHead decoder (attention GRU + big fc) on 8 TRN2 NeuronCores.

Strategy: pure data-parallel over batch (B=256 -> 32/core, no collectives).
Inside each core everything recurrent runs in a "transposed" layout
(feature on partitions, t-major (t*32+b) on the free axis) so per-feature
biases are per-partition and the softmax/context reductions are strided
free-axis reductions.  All TensorE math is bf16 (rel-err gate is loose),
gates/softmax accumulate in f32.  The big fc [800,512]@[512,6625] and all
loop-invariant projections are hoisted out of the 25-step recurrence.
"""

import sys

for _p in ("/opt/trn_rl_repo",):
    if _p not in sys.path:
        sys.path.insert(0, _p)

from contextlib import ExitStack

import ml_dtypes
import numpy as np

import concourse.bass as bass
import concourse.tile as tile
from concourse import bacc, mybir
from concourse.bass_utils import run_bass_kernel_spmd
from concourse.masks import make_identity

B, T, X, S, A, Y, EMB = 256, 25, 512, 512, 512, 6625, 300
NCORES, BL = 8, 32
BT = T * BL            # 800
G = 3 * S              # 1536
KX, KS, KA = X // 128, S // 128, A // 128   # 4 each
KG = G // 128          # 12
EMBP = 384             # EMB padded to 3*128
NY = [512] * 12 + [Y - 12 * 512]   # fc N-splits (481 last)
NBT = ((0, 512), (512, 288))       # 800 -> 512 + 288

bf = mybir.dt.bfloat16
f32 = mybir.dt.float32
i32 = mybir.dt.int32
bfnp = ml_dtypes.bfloat16

TRACE = False
LAST_EXEC_NS = None
_CACHE = {}

AF = mybir.ActivationFunctionType
OP = mybir.AluOpType
AX = mybir.AxisListType


def _build():
    nc = bacc.Bacc("TRN2", target_bir_lowering=False, debug=False,
                   num_devices=NCORES)

    def dp(name, sh, dt):
        return nc.dram_tensor(name, list(sh), dt, kind="ExternalInput").ap()

    xT = dp("xT", (X, BT), bf)            # [x, t*BL+b]
    xft = dp("xft", (T * X, BL), bf)      # [t*X+x, b]
    yidx = dp("yidx", (BT,), i32)         # t-major decoder ids
    eEw = dp("eEw", (T * X, EMB), bf)     # eEmbed_w.T
    wfc = dp("wfc", (EMBP, S), bf)        # embed_fc_w.T zero-padded
    sEw = dp("sEw", (S, A), bf)
    xEw = dp("xEw", (X, A), bf)
    wEw = dp("wEw", (A, 1), bf)
    temb = dp("temb", (Y + 1, A), bf)
    wihy = dp("wihy", (A, G), bf)
    wihc = dp("wihc", (X, G), bf)
    whh = dp("whh", (S, G), bf)
    fcw = dp("fcw", (S, Y), bf)
    fcb = dp("fcb", (Y,), bf)
    sb0 = dp("sb0", (S,), f32)
    sxb = dp("sxb", (A,), f32)
    bgiy = dp("bgiy", (G,), f32)
    bhn = dp("bhn", (S,), f32)
    out = nc.dram_tensor("out", [BL, T, Y], f32, kind="ExternalOutput").ap()

    with tile.TileContext(nc) as tc, ExitStack() as top:
        # ---------------- persistent (whole-kernel) pool -----------------
        pers = top.enter_context(tc.tile_pool(name="pers", bufs=1))
        ident = pers.tile([128, 128], bf, tag="ident")
        make_identity(nc, ident[:])
        ones1 = pers.tile([1, 128], bf, tag="ones1")
        nc.gpsimd.memset(ones1[:], 1.0)

        sb0_t = pers.tile([128, KS], f32, tag="sb0")
        nc.sync.dma_start(sb0_t[:], sb0.rearrange("(j p) -> p j", p=128))
        sxb_t = pers.tile([128, KA], f32, tag="sxb")
        nc.sync.dma_start(sxb_t[:], sxb.rearrange("(j p) -> p j", p=128))
        bgiy_t = pers.tile([128, KG], f32, tag="bgiy")
        nc.sync.dma_start(bgiy_t[:], bgiy.rearrange("(j p) -> p j", p=128))
        bhn_t = pers.tile([128, KS], f32, tag="bhn")
        nc.sync.dma_start(bhn_t[:], bhn.rearrange("(j p) -> p j", p=128))
        wEw_t = pers.tile([128, KA], bf, tag="wEw")
        nc.sync.dma_start(wEw_t.rearrange("p (j o) -> p j o", j=KA),
                          wEw.rearrange("(j p) o -> p j o", p=128))

        stT = pers.tile([128, KS * BL], f32, tag="stT")        # state master
        states = pers.tile([128, KS * T * BL], bf, tag="states")

        # ------------- mid-lifetime pool (phases 2..loop) -------------
        with ExitStack() as mid:
            midp = mid.enter_context(tc.tile_pool(name="midp", bufs=1))
            sEw_t = midp.tile([128, KS * A], bf, tag="sEw")
            nc.sync.dma_start(sEw_t.rearrange("p (j a) -> p j a", j=KS),
                              sEw.rearrange("(j p) a -> p j a", p=128))
            wihc_t = midp.tile([128, KX * G], bf, tag="wihc")
            nc.sync.dma_start(wihc_t.rearrange("p (j g) -> p j g", j=KX),
                              wihc.rearrange("(j p) g -> p j g", p=128))
            whh_t = midp.tile([128, KS * G], bf, tag="whh")
            nc.sync.dma_start(whh_t.rearrange("p (j g) -> p j g", j=KS),
                              whh.rearrange("(j p) g -> p j g", p=128))
            xT_sb = [midp.tile([128, BT], bf, tag=f"xT{j}", name=f"xT{j}")
                     for j in range(KX)]
            for j in range(KX):
                nc.sync.dma_start(xT_sb[j][:], xT[j * 128:(j + 1) * 128, :])
            xP_sb = [midp.tile([128, BT], bf, tag=f"xP{j}", name=f"xP{j}")
                     for j in range(KA)]
            giy = midp.tile([128, KG * T * BL], bf, tag="giy")

            # ---------------- phase 1: embed -> state0 ----------------
            with ExitStack() as ph:
                p_xft = ph.enter_context(tc.tile_pool(name="p_xft", bufs=1))
                p_ew = ph.enter_context(tc.tile_pool(name="p_ew", bufs=6))
                p_ps = ph.enter_context(tc.tile_pool(name="p_ps", bufs=1, space="PSUM"))
                p_tmp = ph.enter_context(tc.tile_pool(name="p_tmp", bufs=1))
                p_pst = ph.enter_context(
                    tc.tile_pool(name="p_pst", bufs=2, space="PSUM"))

                xft_t = p_xft.tile([128, 100 * BL], bf)
                nc.sync.dma_start(xft_t.rearrange("p (k b) -> p k b", k=100),
                                  xft.rearrange("(k p) b -> p k b", p=128))
                ps_em = p_ps.tile([32, EMB], f32, tag="em")
                for k in range(100):
                    ew = p_ew.tile([128, EMB], bf, tag="ew")
                    nc.sync.dma_start(ew[:], eEw[k * 128:(k + 1) * 128, :])
                    nc.tensor.matmul(ps_em[:], xft_t[:, k * 32:(k + 1) * 32], ew[:],
                                     start=(k == 0), stop=(k == 99))
                embed = p_tmp.tile([32, EMB], bf, tag="embed")
                nc.scalar.copy(embed[:], ps_em[:])
                embedT = p_tmp.tile([128, 3 * 32], bf, tag="embedT")
                nc.vector.memset(embedT[:], 0.0)
                for j in range(3):
                    cnt = min(128, EMB - j * 128)
                    pst = p_pst.tile([128, 32], bf, tag="pst")
                    nc.tensor.transpose(pst[:cnt, :],
                                        embed[:, j * 128:j * 128 + cnt],
                                        ident[:32, :32])
                    nc.scalar.copy(embedT[:cnt, j * 32:(j + 1) * 32], pst[:cnt, :])
                wfc_t = p_tmp.tile([128, 3 * S], bf, tag="wfc")
                nc.sync.dma_start(wfc_t.rearrange("p (j a) -> p j a", j=3),
                                  wfc.rearrange("(j p) a -> p j a", p=128))
                ps_s0 = p_ps.tile([128, KS * BL], f32, tag="s0")
                for m in range(KS):
                    for j in range(3):
                        nc.tensor.matmul(
                            ps_s0[:, m * 32:(m + 1) * 32],
                            wfc_t[:, j * S + m * 128:j * S + (m + 1) * 128],
                            embedT[:, j * 32:(j + 1) * 32],
                            start=(j == 0), stop=(j == 2))
                for m in range(KS):
                    nc.scalar.activation(stT[:, m * 32:(m + 1) * 32],
                                         ps_s0[:, m * 32:(m + 1) * 32],
                                         AF.Identity, bias=sb0_t[:, m:m + 1])

            # ---------------- phase 2: xProjT ----------------
            with ExitStack() as ph:
                p_w = ph.enter_context(tc.tile_pool(name="p_xw", bufs=1))
                p_ps = ph.enter_context(
                    tc.tile_pool(name="p_xps", bufs=2, space="PSUM"))
                xEw_t = p_w.tile([128, KX * A], bf)
                nc.sync.dma_start(xEw_t.rearrange("p (j a) -> p j a", j=KX),
                                  xEw.rearrange("(j p) a -> p j a", p=128))
                for m in range(KA):
                    ps = p_ps.tile([128, BT], f32, tag="xp")
                    for (n0, nn) in NBT:
                        for j in range(KX):
                            nc.tensor.matmul(
                                ps[:, n0:n0 + nn],
                                xEw_t[:, j * A + m * 128:j * A + (m + 1) * 128],
                                xT_sb[j][:, n0:n0 + nn],
                                start=(j == 0), stop=(j == KX - 1))
                    nc.scalar.activation(xP_sb[m][:], ps[:], AF.Identity,
                                         bias=sxb_t[:, m:m + 1])

            # ---------------- phase 3: gather tgt emb -> giy ----------------
            with ExitStack() as ph:
                p_idx = ph.enter_context(tc.tile_pool(name="p_idx", bufs=1))
                p_yp = ph.enter_context(tc.tile_pool(name="p_yp", bufs=2))
                p_yT = ph.enter_context(tc.tile_pool(name="p_yT", bufs=1))
                p_w = ph.enter_context(tc.tile_pool(name="p_yw", bufs=1))
                p_pst = ph.enter_context(
                    tc.tile_pool(name="p_ypst", bufs=2, space="PSUM"))
                p_ps = ph.enter_context(
                    tc.tile_pool(name="p_yps", bufs=2, space="PSUM"))

                idx_t = p_idx.tile([128, 7], i32)
                for gch in range(7):
                    cnt = min(128, BT - gch * 128)
                    nc.sync.dma_start(
                        idx_t[:cnt, gch:gch + 1],
                        yidx.rearrange("(p o) -> p o", o=1)[gch * 128:gch * 128 + cnt, :])
                yT = [p_yT.tile([128, BT], bf, tag=f"yT{j}", name=f"yT{j}")
                      for j in range(KA)]
                for gch in range(7):
                    cnt = min(128, BT - gch * 128)
                    yp = p_yp.tile([128, A], bf, tag="yp")
                    nc.gpsimd.indirect_dma_start(
                        out=yp[:cnt, :], out_offset=None, in_=temb[:, :],
                        in_offset=bass.IndirectOffsetOnAxis(
                            ap=idx_t[:cnt, gch:gch + 1], axis=0))
                    for xc in range(KA):
                        pst = p_pst.tile([128, 128], bf, tag="ypst")
                        nc.tensor.transpose(pst[:, :cnt],
                                            yp[:cnt, xc * 128:(xc + 1) * 128],
                                            ident[:cnt, :cnt])
                        nc.scalar.copy(yT[xc][:, gch * 128:gch * 128 + cnt],
                                       pst[:, :cnt])
                wihy_t = p_w.tile([128, KA * G], bf)
                nc.sync.dma_start(wihy_t.rearrange("p (j g) -> p j g", j=KA),
                                  wihy.rearrange("(j p) g -> p j g", p=128))
                for j in range(KG):
                    ps = p_ps.tile([128, BT], f32, tag="gy")
                    for (n0, nn) in NBT:
                        for kj in range(KA):
                            nc.tensor.matmul(
                                ps[:, n0:n0 + nn],
                                wihy_t[:, kj * G + j * 128:kj * G + (j + 1) * 128],
                                yT[kj][:, n0:n0 + nn],
                                start=(kj == 0), stop=(kj == KA - 1))
                    nc.scalar.activation(giy[:, j * BT:(j + 1) * BT], ps[:],
                                         AF.Identity, bias=bgiy_t[:, j:j + 1])

            # ---------------- recurrent loop ----------------
            with ExitStack() as ph:
                ps_sp_p = ph.enter_context(
                    tc.tile_pool(name="ps_sp", bufs=1, space="PSUM"))
                ps_e_p = ph.enter_context(
                    tc.tile_pool(name="ps_e", bufs=1, space="PSUM"))
                ps_ar_p = ph.enter_context(
                    tc.tile_pool(name="ps_ar", bufs=1, space="PSUM"))
                ps_rc_p = ph.enter_context(
                    tc.tile_pool(name="ps_rc", bufs=1, space="PSUM"))
                ps_gi_p = ph.enter_context(
                    tc.tile_pool(name="ps_gi", bufs=1, space="PSUM"))
                ps_gh_p = ph.enter_context(
                    tc.tile_pool(name="ps_gh", bufs=1, space="PSUM"))
                lp = ph.enter_context(tc.tile_pool(name="lp", bufs=2))
                thp = ph.enter_context(tc.tile_pool(name="thp", bufs=4))
                lps = ph.enter_context(tc.tile_pool(name="lps", bufs=3))
                stp = ph.enter_context(tc.tile_pool(name="stp", bufs=2))

                stT_bf = stp.tile([128, KS * BL], bf, tag="stT_bf")
                nc.vector.tensor_copy(stT_bf[:], stT[:])

                for t in range(T):
                    # sProjT: psum cols = a-chunk m
                    ps_sp = ps_sp_p.tile([128, KA * BL], f32, tag="sp")
                    for m in range(KA):
                        for j in range(KS):
                            nc.tensor.matmul(
                                ps_sp[:, m * 32:(m + 1) * 32],
                                sEw_t[:, j * A + m * 128:j * A + (m + 1) * 128],
                                stT_bf[:, j * 32:(j + 1) * 32],
                                start=(j == 0), stop=(j == KS - 1))
                    # tanh(sProj + xProj)
                    spT = lps.tile([128, KA * BL], bf, tag="spT")
                    nc.scalar.copy(spT[:], ps_sp[:])
                    ths = []
                    for m in range(KA):
                        ti = lp.tile([128, BT], bf, tag="ti")
                        nc.vector.tensor_tensor(
                            ti.rearrange("p (t b) -> p t b", t=T),
                            xP_sb[m].rearrange("p (t b) -> p t b", t=T),
                            spT[:, m * 32:(m + 1) * 32]
                                .rearrange("p (o b) -> p o b", o=1).to_broadcast([128, T, BL]),
                            op=OP.add)
                        th = thp.tile([128, BT], bf, tag="th")
                        nc.scalar.activation(th[:], ti[:], AF.Tanh)
                        ths.append(th)
                    # e = w . tanh  -> [1, 800]
                    ps_e = ps_e_p.tile([1, BT], f32, tag="e")
                    for (n0, nn) in NBT:
                        for m in range(KA):
                            nc.tensor.matmul(ps_e[:, n0:n0 + nn],
                                             wEw_t[:, m:m + 1],
                                             ths[m][:, n0:n0 + nn],
                                             start=(m == 0), stop=(m == KA - 1))
                    # softmax over t (no max-sub; |e| is small).  Uses
                    # UNNORMALIZED exp weights; 1/sum folded into ctx cast.
                    exb = lps.tile([1, BT], bf, tag="exb")
                    nc.scalar.activation(exb[:], ps_e[:], AF.Exp)
                    sm = lps.tile([1, BL], f32, tag="sm")
                    nc.vector.reduce_sum(sm[:],
                                         exb.rearrange("p (t b) -> p b t", t=T),
                                         axis=AX.X)
                    rcf = lps.tile([1, BL], f32, tag="rcf")
                    nc.vector.reciprocal(rcf[:], sm[:])
                    rc = lps.tile([1, BL], bf, tag="rc")
                    nc.vector.tensor_copy(rc[:], rcf[:])
                    # broadcast exp weights and 1/sum to 128 partitions via PE
                    ps_ar = ps_ar_p.tile([128, BT], f32, tag="ar")
                    for (n0, nn) in NBT:
                        nc.tensor.matmul(ps_ar[:, n0:n0 + nn], ones1[:, :],
                                         exb[:, n0:n0 + nn],
                                         start=True, stop=True)
                    ps_rc = ps_rc_p.tile([128, BL], f32, tag="rcr")
                    nc.tensor.matmul(ps_rc[:], ones1[:, :], rc[:],
                                     start=True, stop=True)
                    arb = lp.tile([128, BT], bf, tag="arb")
                    nc.scalar.copy(arb[:], ps_ar[:])
                    # context (unnormalized), then normalize during bf16 cast
                    ctxf = lps.tile([128, KX * BL], f32, tag="ctxf")
                    for xc in range(KX):
                        mt = lp.tile([128, BT], bf, tag="mt")
                        nc.vector.tensor_tensor(mt[:], xT_sb[xc][:], arb[:],
                                                op=OP.mult)
                        nc.vector.reduce_sum(
                            ctxf[:, xc * 32:(xc + 1) * 32],
                            mt.rearrange("p (t b) -> p b t", t=T), axis=AX.X)
                    ctxb = lps.tile([128, KX * BL], bf, tag="ctxb")
                    nc.vector.tensor_tensor(
                        ctxb.rearrange("p (j b) -> p j b", j=KX),
                        ctxf.rearrange("p (j b) -> p j b", j=KX),
                        ps_rc.rearrange("p (o b) -> p o b", o=1)
                            .to_broadcast([128, KX, BL]),
                        op=OP.mult)
                    # gi / gh
                    ps_gi = ps_gi_p.tile([128, KG * BL], f32, tag="gi")
                    ps_gh = ps_gh_p.tile([128, KG * BL], f32, tag="gh")
                    for j in range(KG):
                        for kj in range(KX):
                            nc.tensor.matmul(
                                ps_gi[:, j * 32:(j + 1) * 32],
                                wihc_t[:, kj * G + j * 128:kj * G + (j + 1) * 128],
                                ctxb[:, kj * 32:(kj + 1) * 32],
                                start=(kj == 0), stop=(kj == KX - 1))
                    for j in range(KG):
                        for kj in range(KS):
                            nc.tensor.matmul(
                                ps_gh[:, j * 32:(j + 1) * 32],
                                whh_t[:, kj * G + j * 128:kj * G + (j + 1) * 128],
                                stT_bf[:, kj * 32:(kj + 1) * 32],
                                start=(kj == 0), stop=(kj == KS - 1))
                    # gates (f32)
                    ga = lps.tile([128, KG * BL], f32, tag="ga")
                    nc.vector.tensor_tensor(
                        ga.rearrange("p (j b) -> p j b", j=KG),
                        ps_gi.rearrange("p (j b) -> p j b", j=KG),
                        giy.rearrange("p (j t b) -> p j t b", j=KG, t=T)[:, :, t, :],
                        op=OP.add)
                    gb = lps.tile([128, 256], f32, tag="gb")
                    nc.vector.tensor_tensor(gb[:], ga[:, :256], ps_gh[:, :256],
                                            op=OP.add)
                    rz = lps.tile([128, 256], f32, tag="rz")
                    nc.scalar.activation(rz[:], gb[:], AF.Sigmoid)
                    ghn = lps.tile([128, 128], f32, tag="ghn")
                    nc.vector.tensor_tensor(
                        ghn.rearrange("p (j b) -> p j b", j=KS),
                        ps_gh[:, 256:384].rearrange("p (j b) -> p j b", j=KS),
                        bhn_t.rearrange("p (j o) -> p j o", o=1).to_broadcast([128, KS, BL]),
                        op=OP.add)
                    cc = lps.tile([128, 128], f32, tag="cc")
                    nc.vector.tensor_tensor(cc[:], rz[:, :128], ghn[:], op=OP.mult)
                    dd = lps.tile([128, 128], f32, tag="dd")
                    nc.vector.tensor_tensor(dd[:], ga[:, 256:384], cc[:], op=OP.add)
                    nn_ = lps.tile([128, 128], f32, tag="nn")
                    nc.scalar.activation(nn_[:], dd[:], AF.Tanh)
                    ee = lps.tile([128, 128], f32, tag="ee")
                    nc.vector.tensor_tensor(ee[:], stT[:], nn_[:], op=OP.subtract)
                    ff = lps.tile([128, 128], f32, tag="ff")
                    nc.vector.tensor_tensor(ff[:], rz[:, 128:256], ee[:], op=OP.mult)
                    nc.vector.tensor_tensor(stT[:], nn_[:], ff[:], op=OP.add)
                    stT_bf = stp.tile([128, KS * BL], bf, tag="stT_bf")
                    nc.vector.tensor_copy(stT_bf[:], stT[:])
                    nc.vector.tensor_copy(
                        states.rearrange("p (j t b) -> p j t b",
                                         j=KS, t=T)[:, :, t, :],
                        stT.rearrange("p (j b) -> p j b", j=KS))

        # ---------------- fc ----------------
        with ExitStack() as ph:
            p_w = ph.enter_context(tc.tile_pool(name="p_fcw", bufs=1))
            p_ps = ph.enter_context(tc.tile_pool(name="p_fcps", bufs=4, space="PSUM"))
            p_o = ph.enter_context(tc.tile_pool(name="p_fco", bufs=2))
            p_b = ph.enter_context(tc.tile_pool(name="p_fcb", bufs=1))
            p_pb = ph.enter_context(tc.tile_pool(name="p_fcpb", bufs=2, space="PSUM"))

            fcw_sb = [p_w.tile([128, Y], bf, tag=f"fcw{j}", name=f"fcw{j}")
                      for j in range(KS)]
            for j in range(KS):
                nc.sync.dma_start(fcw_sb[j][:], fcw[j * 128:(j + 1) * 128, :])
            fcb1 = p_b.tile([1, Y], bf, tag="fcb1")
            nc.sync.dma_start(fcb1[:], fcb.rearrange("(o y) -> o y", o=1))
            fcbr = p_b.tile([128, Y], bf, tag="fcbr")
            y0 = 0
            for nn in NY:
                pb = p_pb.tile([128, 512], f32, tag="pb")
                nc.tensor.matmul(pb[:, :nn], ones1[:, :], fcb1[:, y0:y0 + nn],
                                 start=True, stop=True)
                nc.vector.tensor_copy(fcbr[:, y0:y0 + nn], pb[:, :nn])
                y0 += nn

            st_f = states.rearrange("p (j tb) -> p j tb", j=KS)
            for mt in range(7):
                cnt = min(128, BT - mt * 128)
                os = p_o.tile([128, Y], f32, tag="os")
                y0 = 0
                for nn in NY:
                    ps = p_ps.tile([128, 512], f32, tag="fps")
                    for kj in range(KS):
                        nc.tensor.matmul(
                            ps[:cnt, :nn],
                            st_f[:, kj, mt * 128:mt * 128 + cnt],
                            fcw_sb[kj][:, y0:y0 + nn],
                            start=(kj == 0), stop=(kj == KS - 1))
                    nc.vector.tensor_tensor(os[:cnt, y0:y0 + nn], ps[:cnt, :nn],
                                            fcbr[:cnt, y0:y0 + nn], op=OP.add)
                    y0 += nn
                for tt in range(cnt // 32):
                    nc.sync.dma_start(out[:, 4 * mt + tt, :],
                                      os[tt * 32:(tt + 1) * 32, :])

    nc.compile()
    return nc


def _host_prep(inputs):
    x = np.asarray(inputs["x"], np.float32)
    tg = np.asarray(inputs["targets"]).astype(np.int64)
    assert int(inputs["lengths"]) == T, "kernel hardcodes T_dec=25"
    w = {k: np.asarray(v, np.float32) for k, v in inputs.items()
         if k not in ("x", "targets", "lengths")}

    y_seq = np.full((T, B), Y, np.int64)
    y_seq[1:] = tg[:, :T - 1].T
    y_seq = y_seq.astype(np.int32)

    wfc_pad = np.zeros((EMBP, S), np.float32)
    wfc_pad[:EMB] = w["embed_fc_w"].T
    shared = {
        "eEw": np.ascontiguousarray(w["eEmbed_w"].T).astype(bfnp),
        "wfc": wfc_pad.astype(bfnp),
        "sEw": np.ascontiguousarray(w["sEmbed_w"].T).astype(bfnp),
        "xEw": np.ascontiguousarray(w["xEmbed_w"].T).astype(bfnp),
        "wEw": np.ascontiguousarray(w["wEmbed_w"].T).astype(bfnp),
        "temb": np.ascontiguousarray(w["tgt_emb"]).astype(bfnp),
        "wihy": np.ascontiguousarray(w["gru_w_ih"][:, :A].T).astype(bfnp),
        "wihc": np.ascontiguousarray(w["gru_w_ih"][:, A:].T).astype(bfnp),
        "whh": np.ascontiguousarray(w["gru_w_hh"].T).astype(bfnp),
        "fcw": np.ascontiguousarray(w["fc_w"].T).astype(bfnp),
        "fcb": w["fc_b"].astype(bfnp),
        "sb0": (w["embed_fc_w"] @ w["eEmbed_b"] + w["embed_fc_b"]).astype(np.float32),
        "sxb": (w["sEmbed_b"] + w["xEmbed_b"]).astype(np.float32),
        "bgiy": (w["gru_b_ih"]
                 + np.concatenate([w["gru_b_hh"][:2 * S],
                                   np.zeros(S, np.float32)])).astype(np.float32),
        "bhn": w["gru_b_hh"][2 * S:].astype(np.float32),
    }
    in_maps = []
    for c in range(NCORES):
        xc = x[c * BL:(c + 1) * BL]                       # [32, 25, 512]
        m = dict(shared)
        m["xT"] = np.ascontiguousarray(
            xc.transpose(2, 1, 0).reshape(X, BT)).astype(bfnp)
        m["xft"] = np.ascontiguousarray(
            xc.transpose(1, 2, 0).reshape(T * X, BL)).astype(bfnp)
        m["yidx"] = np.ascontiguousarray(y_seq[:, c * BL:(c + 1) * BL].reshape(BT))
        in_maps.append(m)
    return in_maps


def kernel(**inputs):
    global LAST_EXEC_NS
    if "nc" not in _CACHE:
        _CACHE["nc"] = _build()
    nc = _CACHE["nc"]
    in_maps = _host_prep(inputs)
    res = run_bass_kernel_spmd(nc, in_maps, core_ids=list(range(NCORES)),
                               trace=TRACE)
    LAST_EXEC_NS = res.exec_time_ns
    outs = [res.results[c]["out"] for c in range(NCORES)]
    return np.concatenate(outs, axis=0)


# revision 27
# speedup vs baseline: 1.5052x; 1.1646x over previous
"""AsterHead decoder (attention GRU + big fc) on 8 TRN2 NeuronCores.

Strategy: pure data-parallel over batch (B=256 -> 32/core, no collectives).
Inside each core everything recurrent runs in a "transposed" layout
(feature on partitions, t-major (t*32+b) on the free axis) so per-feature
biases are per-partition and the softmax/context reductions are strided
free-axis reductions.  All TensorE math is bf16 (rel-err gate is loose),
gates/softmax accumulate in f32.  The big fc [800,512]@[512,6625] and all
loop-invariant projections are hoisted out of the 25-step recurrence.
"""

import sys

for _p in ("/opt/trn_rl_repo",):
    if _p not in sys.path:
        sys.path.insert(0, _p)

from contextlib import ExitStack

import ml_dtypes
import numpy as np

import concourse.bass as bass
import concourse.tile as tile
from concourse import bacc, mybir
from concourse.bass_utils import run_bass_kernel_spmd
from concourse.masks import make_identity

B, T, X, S, A, Y, EMB = 256, 25, 512, 512, 512, 6625, 300
NCORES, BL = 8, 32
BT = T * BL            # 800
G = 3 * S              # 1536
KX, KS, KA = X // 128, S // 128, A // 128   # 4 each
KG = G // 128          # 12
EMBP = 384             # EMB padded to 3*128
NY = [512] * 12 + [Y - 12 * 512]   # fc N-splits (481 last)
NBT = ((0, 512), (512, 288))       # 800 -> 512 + 288

bf = mybir.dt.bfloat16
f32 = mybir.dt.float32
i32 = mybir.dt.int32
bfnp = ml_dtypes.bfloat16

TRACE = False
LAST_EXEC_NS = None
_CACHE = {}

AF = mybir.ActivationFunctionType
OP = mybir.AluOpType
AX = mybir.AxisListType


def _build():
    nc = bacc.Bacc("TRN2", target_bir_lowering=False, debug=False,
                   num_devices=NCORES)

    def dp(name, sh, dt):
        return nc.dram_tensor(name, list(sh), dt, kind="ExternalInput").ap()

    xT = dp("xT", (X, BT), bf)            # [x, t*BL+b]
    xft = dp("xft", (T * X, BL), bf)      # [t*X+x, b]
    yidx = dp("yidx", (BT,), i32)         # t-major decoder ids
    eEw = dp("eEw", (T * X, EMB), bf)     # eEmbed_w.T
    wfc = dp("wfc", (EMBP, S), bf)        # embed_fc_w.T zero-padded
    sEw = dp("sEw", (S, A), bf)
    xEw = dp("xEw", (X, A), bf)
    wEw = dp("wEw", (A, 1), bf)
    temb = dp("temb", (Y + 1, A), bf)
    wihy = dp("wihy", (A, G), bf)
    wihc = dp("wihc", (X, G), bf)
    whh = dp("whh", (S, G), bf)
    fcw = dp("fcw", (S, Y), bf)
    fcb = dp("fcb", (Y,), bf)
    sb0 = dp("sb0", (S,), f32)
    sxb = dp("sxb", (A,), f32)
    bgiy = dp("bgiy", (G,), f32)
    bhn = dp("bhn", (S,), f32)
    out = nc.dram_tensor("out", [BL, T, Y], f32, kind="ExternalOutput").ap()

    with tile.TileContext(nc) as tc, ExitStack() as top:
        # ---------------- persistent (whole-kernel) pool -----------------
        pers = top.enter_context(tc.tile_pool(name="pers", bufs=1))
        ident = pers.tile([128, 128], bf, tag="ident")
        make_identity(nc, ident[:])
        ones1 = pers.tile([1, 128], bf, tag="ones1")
        nc.gpsimd.memset(ones1[:], 1.0)

        sb0_t = pers.tile([128, KS], f32, tag="sb0")
        nc.sync.dma_start(sb0_t[:], sb0.rearrange("(j p) -> p j", p=128))
        sxb_t = pers.tile([128, KA], f32, tag="sxb")
        nc.sync.dma_start(sxb_t[:], sxb.rearrange("(j p) -> p j", p=128))
        bgiy_t = pers.tile([128, KG], f32, tag="bgiy")
        nc.sync.dma_start(bgiy_t[:], bgiy.rearrange("(j p) -> p j", p=128))
        bhn_t = pers.tile([128, KS], f32, tag="bhn")
        nc.sync.dma_start(bhn_t[:], bhn.rearrange("(j p) -> p j", p=128))
        wEw_t = pers.tile([128, KA], bf, tag="wEw")
        nc.sync.dma_start(wEw_t.rearrange("p (j o) -> p j o", j=KA),
                          wEw.rearrange("(j p) o -> p j o", p=128))

        stT = pers.tile([128, KS * BL], f32, tag="stT")        # state master
        states = pers.tile([128, KS * T * BL], bf, tag="states")

        # ------------- mid-lifetime pool (phases 2..loop) -------------
        with ExitStack() as mid:
            midp = mid.enter_context(tc.tile_pool(name="midp", bufs=1))
            sEw_t = midp.tile([128, KS * A], bf, tag="sEw")
            nc.sync.dma_start(sEw_t.rearrange("p (j a) -> p j a", j=KS),
                              sEw.rearrange("(j p) a -> p j a", p=128))
            wihc_t = midp.tile([128, KX * G], bf, tag="wihc")
            nc.sync.dma_start(wihc_t.rearrange("p (j g) -> p j g", j=KX),
                              wihc.rearrange("(j p) g -> p j g", p=128))
            whh_t = midp.tile([128, KS * G], bf, tag="whh")
            nc.sync.dma_start(whh_t.rearrange("p (j g) -> p j g", j=KS),
                              whh.rearrange("(j p) g -> p j g", p=128))
            xT_sb = [midp.tile([128, BT], bf, tag=f"xT{j}", name=f"xT{j}")
                     for j in range(KX)]
            for j in range(KX):
                nc.sync.dma_start(xT_sb[j][:], xT[j * 128:(j + 1) * 128, :])
            xP_sb = [midp.tile([128, BT], bf, tag=f"xP{j}", name=f"xP{j}")
                     for j in range(KA)]
            giy = midp.tile([128, KG * T * BL], bf, tag="giy")

            # ---------------- phase 1: embed -> state0 ----------------
            with ExitStack() as ph:
                p_xft = ph.enter_context(tc.tile_pool(name="p_xft", bufs=1))
                p_ew = ph.enter_context(tc.tile_pool(name="p_ew", bufs=6))
                p_ps = ph.enter_context(tc.tile_pool(name="p_ps", bufs=1, space="PSUM"))
                p_tmp = ph.enter_context(tc.tile_pool(name="p_tmp", bufs=1))
                p_pst = ph.enter_context(
                    tc.tile_pool(name="p_pst", bufs=2, space="PSUM"))

                xft_t = p_xft.tile([128, 100 * BL], bf)
                nc.sync.dma_start(xft_t.rearrange("p (k b) -> p k b", k=100),
                                  xft.rearrange("(k p) b -> p k b", p=128))
                ps_em = p_ps.tile([32, EMB], f32, tag="em")
                for k in range(100):
                    ew = p_ew.tile([128, EMB], bf, tag="ew")
                    nc.sync.dma_start(ew[:], eEw[k * 128:(k + 1) * 128, :])
                    nc.tensor.matmul(ps_em[:], xft_t[:, k * 32:(k + 1) * 32], ew[:],
                                     start=(k == 0), stop=(k == 99))
                embed = p_tmp.tile([32, EMB], bf, tag="embed")
                nc.scalar.copy(embed[:], ps_em[:])
                embedT = p_tmp.tile([128, 3 * 32], bf, tag="embedT")
                nc.vector.memset(embedT[:], 0.0)
                for j in range(3):
                    cnt = min(128, EMB - j * 128)
                    pst = p_pst.tile([128, 32], bf, tag="pst")
                    nc.tensor.transpose(pst[:cnt, :],
                                        embed[:, j * 128:j * 128 + cnt],
                                        ident[:32, :32])
                    nc.scalar.copy(embedT[:cnt, j * 32:(j + 1) * 32], pst[:cnt, :])
                wfc_t = p_tmp.tile([128, 3 * S], bf, tag="wfc")
                nc.sync.dma_start(wfc_t.rearrange("p (j a) -> p j a", j=3),
                                  wfc.rearrange("(j p) a -> p j a", p=128))
                ps_s0 = p_ps.tile([128, KS * BL], f32, tag="s0")
                for m in range(KS):
                    for j in range(3):
                        nc.tensor.matmul(
                            ps_s0[:, m * 32:(m + 1) * 32],
                            wfc_t[:, j * S + m * 128:j * S + (m + 1) * 128],
                            embedT[:, j * 32:(j + 1) * 32],
                            start=(j == 0), stop=(j == 2))
                for m in range(KS):
                    nc.scalar.activation(stT[:, m * 32:(m + 1) * 32],
                                         ps_s0[:, m * 32:(m + 1) * 32],
                                         AF.Identity, bias=sb0_t[:, m:m + 1])

            # ---------------- phase 2: xProjT ----------------
            with ExitStack() as ph:
                p_w = ph.enter_context(tc.tile_pool(name="p_xw", bufs=1))
                p_ps = ph.enter_context(
                    tc.tile_pool(name="p_xps", bufs=2, space="PSUM"))
                xEw_t = p_w.tile([128, KX * A], bf)
                nc.sync.dma_start(xEw_t.rearrange("p (j a) -> p j a", j=KX),
                                  xEw.rearrange("(j p) a -> p j a", p=128))
                for m in range(KA):
                    ps = p_ps.tile([128, BT], f32, tag="xp")
                    for (n0, nn) in NBT:
                        for j in range(KX):
                            nc.tensor.matmul(
                                ps[:, n0:n0 + nn],
                                xEw_t[:, j * A + m * 128:j * A + (m + 1) * 128],
                                xT_sb[j][:, n0:n0 + nn],
                                start=(j == 0), stop=(j == KX - 1))
                    nc.scalar.activation(xP_sb[m][:], ps[:], AF.Identity,
                                         bias=sxb_t[:, m:m + 1])

            # ---------------- phase 3: gather tgt emb -> giy ----------------
            with ExitStack() as ph:
                p_idx = ph.enter_context(tc.tile_pool(name="p_idx", bufs=1))
                p_yp = ph.enter_context(tc.tile_pool(name="p_yp", bufs=2))
                p_yT = ph.enter_context(tc.tile_pool(name="p_yT", bufs=1))
                p_w = ph.enter_context(tc.tile_pool(name="p_yw", bufs=1))
                p_pst = ph.enter_context(
                    tc.tile_pool(name="p_ypst", bufs=2, space="PSUM"))
                p_ps = ph.enter_context(
                    tc.tile_pool(name="p_yps", bufs=2, space="PSUM"))

                idx_t = p_idx.tile([128, 7], i32)
                for gch in range(7):
                    cnt = min(128, BT - gch * 128)
                    nc.sync.dma_start(
                        idx_t[:cnt, gch:gch + 1],
                        yidx.rearrange("(p o) -> p o", o=1)[gch * 128:gch * 128 + cnt, :])
                yT = [p_yT.tile([128, BT], bf, tag=f"yT{j}", name=f"yT{j}")
                      for j in range(KA)]
                for gch in range(7):
                    cnt = min(128, BT - gch * 128)
                    yp = p_yp.tile([128, A], bf, tag="yp")
                    nc.gpsimd.indirect_dma_start(
                        out=yp[:cnt, :], out_offset=None, in_=temb[:, :],
                        in_offset=bass.IndirectOffsetOnAxis(
                            ap=idx_t[:cnt, gch:gch + 1], axis=0))
                    for xc in range(KA):
                        pst = p_pst.tile([128, 128], bf, tag="ypst")
                        nc.tensor.transpose(pst[:, :cnt],
                                            yp[:cnt, xc * 128:(xc + 1) * 128],
                                            ident[:cnt, :cnt])
                        nc.scalar.copy(yT[xc][:, gch * 128:gch * 128 + cnt],
                                       pst[:, :cnt])
                wihy_t = p_w.tile([128, KA * G], bf)
                nc.sync.dma_start(wihy_t.rearrange("p (j g) -> p j g", j=KA),
                                  wihy.rearrange("(j p) g -> p j g", p=128))
                for j in range(KG):
                    ps = p_ps.tile([128, BT], f32, tag="gy")
                    for (n0, nn) in NBT:
                        for kj in range(KA):
                            nc.tensor.matmul(
                                ps[:, n0:n0 + nn],
                                wihy_t[:, kj * G + j * 128:kj * G + (j + 1) * 128],
                                yT[kj][:, n0:n0 + nn],
                                start=(kj == 0), stop=(kj == KA - 1))
                    nc.scalar.activation(giy[:, j * BT:(j + 1) * BT], ps[:],
                                         AF.Identity, bias=bgiy_t[:, j:j + 1])

            # ---------------- recurrent loop ----------------
            with ExitStack() as ph:
                ps_sp_p = ph.enter_context(
                    tc.tile_pool(name="ps_sp", bufs=1, space="PSUM"))
                ps_e_p = ph.enter_context(
                    tc.tile_pool(name="ps_e", bufs=1, space="PSUM"))
                ps_ar_p = ph.enter_context(
                    tc.tile_pool(name="ps_ar", bufs=1, space="PSUM"))
                ps_rc_p = ph.enter_context(
                    tc.tile_pool(name="ps_rc", bufs=1, space="PSUM"))
                ps_gi_p = ph.enter_context(
                    tc.tile_pool(name="ps_gi", bufs=1, space="PSUM"))
                ps_gh_p = ph.enter_context(
                    tc.tile_pool(name="ps_gh", bufs=1, space="PSUM"))
                lp = ph.enter_context(tc.tile_pool(name="lp", bufs=3))
                thp = ph.enter_context(tc.tile_pool(name="thp", bufs=4))
                lps = ph.enter_context(tc.tile_pool(name="lps", bufs=3))
                stp = ph.enter_context(tc.tile_pool(name="stp", bufs=2))

                stT_bf = stp.tile([128, KS * BL], bf, tag="stT_bf")
                nc.vector.tensor_copy(stT_bf[:], stT[:])

                for t in range(T):
                    # sProjT: psum cols = a-chunk m
                    ps_sp = ps_sp_p.tile([128, KA * BL], f32, tag="sp")
                    for m in range(KA):
                        for j in range(KS):
                            nc.tensor.matmul(
                                ps_sp[:, m * 32:(m + 1) * 32],
                                sEw_t[:, j * A + m * 128:j * A + (m + 1) * 128],
                                stT_bf[:, j * 32:(j + 1) * 32],
                                start=(j == 0), stop=(j == KS - 1))
                    # tanh(sProj + xProj)
                    spT = lps.tile([128, KA * BL], bf, tag="spT")
                    nc.scalar.copy(spT[:], ps_sp[:])
                    ths = []
                    for m in range(KA):
                        ti = lp.tile([128, BT], bf, tag="ti")
                        nc.vector.tensor_tensor(
                            ti.rearrange("p (t b) -> p t b", t=T),
                            xP_sb[m].rearrange("p (t b) -> p t b", t=T),
                            spT[:, m * 32:(m + 1) * 32]
                                .rearrange("p (o b) -> p o b", o=1).to_broadcast([128, T, BL]),
                            op=OP.add)
                        th = thp.tile([128, BT], bf, tag="th")
                        nc.scalar.activation(th[:], ti[:], AF.Tanh)
                        ths.append(th)
                    # e = w . tanh  -> [1, 800]
                    ps_e = ps_e_p.tile([1, BT], f32, tag="e")
                    for (n0, nn) in NBT:
                        for m in range(KA):
                            nc.tensor.matmul(ps_e[:, n0:n0 + nn],
                                             wEw_t[:, m:m + 1],
                                             ths[m][:, n0:n0 + nn],
                                             start=(m == 0), stop=(m == KA - 1))
                    # softmax over t (no max-sub; |e| is small).  Uses
                    # UNNORMALIZED exp weights; 1/sum folded into ctx cast.
                    exb = lps.tile([1, BT], bf, tag="exb")
                    nc.scalar.activation(exb[:], ps_e[:], AF.Exp)
                    sm = lps.tile([1, BL], f32, tag="sm")
                    nc.vector.reduce_sum(sm[:],
                                         exb.rearrange("p (t b) -> p b t", t=T),
                                         axis=AX.X)
                    rcf = lps.tile([1, BL], f32, tag="rcf")
                    nc.vector.reciprocal(rcf[:], sm[:])
                    rc = lps.tile([1, BL], bf, tag="rc")
                    nc.vector.tensor_copy(rc[:], rcf[:])
                    # broadcast exp weights and 1/sum to 128 partitions via PE
                    ps_ar = ps_ar_p.tile([128, BT], f32, tag="ar")
                    for (n0, nn) in NBT:
                        nc.tensor.matmul(ps_ar[:, n0:n0 + nn], ones1[:, :],
                                         exb[:, n0:n0 + nn],
                                         start=True, stop=True)
                    ps_rc = ps_rc_p.tile([128, BL], f32, tag="rcr")
                    nc.tensor.matmul(ps_rc[:], ones1[:, :], rc[:],
                                     start=True, stop=True)
                    arb = lp.tile([128, BT], bf, tag="arb")
                    nc.scalar.copy(arb[:], ps_ar[:])
                    # context (unnormalized), then normalize during bf16 cast
                    ctxf = lps.tile([128, KX * BL], f32, tag="ctxf")
                    for xc in range(KX):
                        mt = lp.tile([128, BT], bf, tag="mt")
                        nc.vector.tensor_tensor(mt[:], xT_sb[xc][:], arb[:],
                                                op=OP.mult)
                        nc.vector.reduce_sum(
                            ctxf[:, xc * 32:(xc + 1) * 32],
                            mt.rearrange("p (t b) -> p b t", t=T), axis=AX.X)
                    ctxb = lps.tile([128, KX * BL], bf, tag="ctxb")
                    nc.vector.tensor_tensor(
                        ctxb.rearrange("p (j b) -> p j b", j=KX),
                        ctxf.rearrange("p (j b) -> p j b", j=KX),
                        ps_rc.rearrange("p (o b) -> p o b", o=1)
                            .to_broadcast([128, KX, BL]),
                        op=OP.mult)
                    # gi / gh
                    ps_gi = ps_gi_p.tile([128, KG * BL], f32, tag="gi")
                    ps_gh = ps_gh_p.tile([128, KG * BL], f32, tag="gh")
                    for j in range(KG):
                        for kj in range(KX):
                            nc.tensor.matmul(
                                ps_gi[:, j * 32:(j + 1) * 32],
                                wihc_t[:, kj * G + j * 128:kj * G + (j + 1) * 128],
                                ctxb[:, kj * 32:(kj + 1) * 32],
                                start=(kj == 0), stop=(kj == KX - 1))
                    for j in range(KG):
                        for kj in range(KS):
                            nc.tensor.matmul(
                                ps_gh[:, j * 32:(j + 1) * 32],
                                whh_t[:, kj * G + j * 128:kj * G + (j + 1) * 128],
                                stT_bf[:, kj * 32:(kj + 1) * 32],
                                start=(kj == 0), stop=(kj == KS - 1))
                    # gates (f32)
                    ga = lps.tile([128, KG * BL], f32, tag="ga")
                    nc.vector.tensor_tensor(
                        ga.rearrange("p (j b) -> p j b", j=KG),
                        ps_gi.rearrange("p (j b) -> p j b", j=KG),
                        giy.rearrange("p (j t b) -> p j t b", j=KG, t=T)[:, :, t, :],
                        op=OP.add)
                    gb = lps.tile([128, 256], f32, tag="gb")
                    nc.vector.tensor_tensor(gb[:], ga[:, :256], ps_gh[:, :256],
                                            op=OP.add)
                    rz = lps.tile([128, 256], f32, tag="rz")
                    nc.scalar.activation(rz[:], gb[:], AF.Sigmoid)
                    ghn = lps.tile([128, 128], f32, tag="ghn")
                    nc.vector.tensor_tensor(
                        ghn.rearrange("p (j b) -> p j b", j=KS),
                        ps_gh[:, 256:384].rearrange("p (j b) -> p j b", j=KS),
                        bhn_t.rearrange("p (j o) -> p j o", o=1).to_broadcast([128, KS, BL]),
                        op=OP.add)
                    cc = lps.tile([128, 128], f32, tag="cc")
                    nc.vector.tensor_tensor(cc[:], rz[:, :128], ghn[:], op=OP.mult)
                    dd = lps.tile([128, 128], f32, tag="dd")
                    nc.vector.tensor_tensor(dd[:], ga[:, 256:384], cc[:], op=OP.add)
                    nn_ = lps.tile([128, 128], f32, tag="nn")
                    nc.scalar.activation(nn_[:], dd[:], AF.Tanh)
                    ee = lps.tile([128, 128], f32, tag="ee")
                    nc.vector.tensor_tensor(ee[:], stT[:], nn_[:], op=OP.subtract)
                    ff = lps.tile([128, 128], f32, tag="ff")
                    nc.vector.tensor_tensor(ff[:], rz[:, 128:256], ee[:], op=OP.mult)
                    nc.vector.tensor_tensor(stT[:], nn_[:], ff[:], op=OP.add)
                    stT_bf = stp.tile([128, KS * BL], bf, tag="stT_bf")
                    nc.vector.tensor_copy(stT_bf[:], stT[:])
                    nc.vector.tensor_copy(
                        states.rearrange("p (j t b) -> p j t b",
                                         j=KS, t=T)[:, :, t, :],
                        stT.rearrange("p (j b) -> p j b", j=KS))

        # ---------------- fc ----------------
        with ExitStack() as ph:
            p_w = ph.enter_context(tc.tile_pool(name="p_fcw", bufs=1))
            p_ps = ph.enter_context(tc.tile_pool(name="p_fcps", bufs=4, space="PSUM"))
            p_o = ph.enter_context(tc.tile_pool(name="p_fco", bufs=2))
            p_b = ph.enter_context(tc.tile_pool(name="p_fcb", bufs=1))
            p_pb = ph.enter_context(tc.tile_pool(name="p_fcpb", bufs=2, space="PSUM"))

            fcw_sb = [p_w.tile([128, Y], bf, tag=f"fcw{j}", name=f"fcw{j}")
                      for j in range(KS)]
            for j in range(KS):
                nc.sync.dma_start(fcw_sb[j][:], fcw[j * 128:(j + 1) * 128, :])
            fcb1 = p_b.tile([1, Y], bf, tag="fcb1")
            nc.sync.dma_start(fcb1[:], fcb.rearrange("(o y) -> o y", o=1))
            fcbr = p_b.tile([128, Y], bf, tag="fcbr")
            y0 = 0
            for nn in NY:
                pb = p_pb.tile([128, 512], f32, tag="pb")
                nc.tensor.matmul(pb[:, :nn], ones1[:, :], fcb1[:, y0:y0 + nn],
                                 start=True, stop=True)
                nc.vector.tensor_copy(fcbr[:, y0:y0 + nn], pb[:, :nn])
                y0 += nn

            st_f = states.rearrange("p (j tb) -> p j tb", j=KS)
            for mt in range(7):
                cnt = min(128, BT - mt * 128)
                os = p_o.tile([128, Y], f32, tag="os")
                y0 = 0
                for nn in NY:
                    ps = p_ps.tile([128, 512], f32, tag="fps")
                    for kj in range(KS):
                        nc.tensor.matmul(
                            ps[:cnt, :nn],
                            st_f[:, kj, mt * 128:mt * 128 + cnt],
                            fcw_sb[kj][:, y0:y0 + nn],
                            start=(kj == 0), stop=(kj == KS - 1))
                    nc.vector.tensor_tensor(os[:cnt, y0:y0 + nn], ps[:cnt, :nn],
                                            fcbr[:cnt, y0:y0 + nn], op=OP.add)
                    y0 += nn
                for tt in range(cnt // 32):
                    nc.sync.dma_start(out[:, 4 * mt + tt, :],
                                      os[tt * 32:(tt + 1) * 32, :])

    nc.compile()
    return nc


def _host_prep(inputs):
    x = np.asarray(inputs["x"], np.float32)
    tg = np.asarray(inputs["targets"]).astype(np.int64)
    assert int(inputs["lengths"]) == T, "kernel hardcodes T_dec=25"
    w = {k: np.asarray(v, np.float32) for k, v in inputs.items()
         if k not in ("x", "targets", "lengths")}

    y_seq = np.full((T, B), Y, np.int64)
    y_seq[1:] = tg[:, :T - 1].T
    y_seq = y_seq.astype(np.int32)

    wfc_pad = np.zeros((EMBP, S), np.float32)
    wfc_pad[:EMB] = w["embed_fc_w"].T
    shared = {
        "eEw": np.ascontiguousarray(w["eEmbed_w"].T).astype(bfnp),
        "wfc": wfc_pad.astype(bfnp),
        "sEw": np.ascontiguousarray(w["sEmbed_w"].T).astype(bfnp),
        "xEw": np.ascontiguousarray(w["xEmbed_w"].T).astype(bfnp),
        "wEw": np.ascontiguousarray(w["wEmbed_w"].T).astype(bfnp),
        "temb": np.ascontiguousarray(w["tgt_emb"]).astype(bfnp),
        "wihy": np.ascontiguousarray(w["gru_w_ih"][:, :A].T).astype(bfnp),
        "wihc": np.ascontiguousarray(w["gru_w_ih"][:, A:].T).astype(bfnp),
        "whh": np.ascontiguousarray(w["gru_w_hh"].T).astype(bfnp),
        "fcw": np.ascontiguousarray(w["fc_w"].T).astype(bfnp),
        "fcb": w["fc_b"].astype(bfnp),
        "sb0": (w["embed_fc_w"] @ w["eEmbed_b"] + w["embed_fc_b"]).astype(np.float32),
        "sxb": (w["sEmbed_b"] + w["xEmbed_b"]).astype(np.float32),
        "bgiy": (w["gru_b_ih"]
                 + np.concatenate([w["gru_b_hh"][:2 * S],
                                   np.zeros(S, np.float32)])).astype(np.float32),
        "bhn": w["gru_b_hh"][2 * S:].astype(np.float32),
    }
    in_maps = []
    for c in range(NCORES):
        xc = x[c * BL:(c + 1) * BL]                       # [32, 25, 512]
        m = dict(shared)
        m["xT"] = np.ascontiguousarray(
            xc.transpose(2, 1, 0).reshape(X, BT)).astype(bfnp)
        m["xft"] = np.ascontiguousarray(
            xc.transpose(1, 2, 0).reshape(T * X, BL)).astype(bfnp)
        m["yidx"] = np.ascontiguousarray(y_seq[:, c * BL:(c + 1) * BL].reshape(BT))
        in_maps.append(m)
    return in_maps


def kernel(**inputs):
    global LAST_EXEC_NS
    if "nc" not in _CACHE:
        _CACHE["nc"] = _build()
    nc = _CACHE["nc"]
    in_maps = _host_prep(inputs)
    res = run_bass_kernel_spmd(nc, in_maps, core_ids=list(range(NCORES)),
                               trace=TRACE)
    LAST_EXEC_NS = res.exec_time_ns
    outs = [res.results[c]["out"] for c in range(NCORES)]
    return np.concatenate(outs, axis=0)
